# revision 7
# baseline (speedup 1.0000x reference)
"""Trainium2 Bass kernel for a feed-forward NTM step (nn_FFNTM).

Strategy (per sharding hint): shard M / rinit / winit / weightings along the
N (memory-location) axis across 8 cores. Each core holds N/8 = 32768 rows of
M resident in SBUF. Content addressing (cosine sim via PE), softmax
(exp-sum partials + AllReduce), interpolation, circular shift (PE matmul with
a tridiagonal shift matrix + boundary patches using 1-row halos), sharpening
(ln/exp), read reduction (PE, partials carried in a second AllReduce) and the
erase/add write (rank-1 PE outer products + fused DVE/GpSimd elementwise) are
all row-parallel. The tiny controller/head linears (O(C^2) work) are computed
replicated on the host in fp32 and fed in as small constant tensors.

Device does all O(N) work; exactly two small AllReduces cross cores.
"""
import sys

for _p in ("/opt/trn_rl_repo", "/root/.axon_site/_ro/trn_rl_repo"):
    if _p not in sys.path:
        sys.path.insert(0, _p)

import numpy as np

# ---- problem dims (hardcoded; must match reference.py) ----
N, D, C, E_IN, OUT, R, WH, S = 262144, 128, 512, 256, 256, 2, 1, 3
NH = R + WH               # 3 heads: read0, read1, write0
NCORES = 8
NC = N // NCORES          # 32768 rows per core
TILE = 512                # rows per SBUF tile group (4 chunks of 128)
NT = NC // TILE           # 64 tiles
NCH = NC // 128           # 256 chunks == w-layout columns per head
EPS = 1e-8
F32 = None                # filled after imports


# ----------------------------------------------------------------- host math
def _softplus(x):
    x = np.asarray(x, np.float32)
    return (np.log1p(np.exp(-np.abs(x))) + np.maximum(x, 0)).astype(np.float32)


def _sigmoid(x):
    return (1.0 / (1.0 + np.exp(-np.asarray(x, np.float32)))).astype(np.float32)


def _softmax(x):
    x = np.asarray(x, np.float32)
    ex = np.exp(x - x.max())
    return (ex / ex.sum()).astype(np.float32)


def host_params(inp):
    g = {k: np.asarray(v, np.float32) for k, v in inp.items()}
    x = np.concatenate([g["external_input"], g["prev_read"].reshape(-1)])
    c = (g["Wc"] @ x + g["bc"]).astype(np.float32)
    heads = []
    for pre, n in (("r", R), ("w", WH)):
        for h in range(n):
            heads.append(dict(
                k=np.tanh(g[pre + "Wk"][h] @ c + g[pre + "bk"][h]).astype(np.float32),
                beta=float(_softplus(g[pre + "Wb"][h] @ c + g[pre + "bb"][h])),
                gg=float(_sigmoid(g[pre + "Wg"][h] @ c + g[pre + "bg"][h])),
                s=_softmax(g[pre + "Ws"][h] @ c + g[pre + "bs"][h]),
                gamma=float(1.0 + _softplus(g[pre + "Wgm"][h] @ c + g[pre + "bgm"][h])),
            ))
    e = _sigmoid(g["wWe"][0] @ c + g["wbe"][0])
    a = np.tanh(g["wWa"][0] @ c + g["wba"][0]).astype(np.float32)
    return c, heads, e, a


# ------------------------------------------------------------- device build
def build(nc, nc_rows=NC, heads=None, e=None, a=None):
    """Emit the SPMD Bass/Tile program. heads/e/a are host-computed params
    (head scalars become instruction immediates; vectors become inputs)."""
    import concourse.bass as bass
    import concourse.mybir as mybir
    import concourse.tile as tile

    f32 = mybir.dt.float32
    f32r = mybir.dt.float32r
    AF = mybir.ActivationFunctionType
    ALU = mybir.AluOpType
    AX = mybir.AxisListType

    nch = nc_rows // 128
    nt = nc_rows // TILE
    hblk = nch                       # w-layout columns per head
    hw2 = hblk // 2

    # ---- DRAM I/O ----
    m_d = nc.dram_tensor("m", [nc_rows, D], f32, kind="ExternalInput").ap()
    mhalo_d = nc.dram_tensor("mhalo", [2, D], f32, kind="ExternalInput").ap()
    init_d = nc.dram_tensor("init", [NH, nc_rows], f32, kind="ExternalInput").ap()
    inithalo_d = nc.dram_tensor("inithalo", [NH, 2], f32, kind="ExternalInput").ap()
    kb_d = nc.dram_tensor("kb", [D, NH], f32, kind="ExternalInput").ap()
    c3_d = nc.dram_tensor("c3", [128, 128 * NH], f32, kind="ExternalInput").ap()
    e4_d = nc.dram_tensor("e4", [128, TILE], f32, kind="ExternalInput").ap()
    eblk_d = nc.dram_tensor("eblk", [4, TILE], f32r, kind="ExternalInput").ap()
    ablk_d = nc.dram_tensor("ablk", [4, TILE], f32r, kind="ExternalInput").ap()
    i128_d = nc.dram_tensor("i128", [128, 128], f32, kind="ExternalInput").ap()
    ones4_d = nc.dram_tensor("ones4", [128, 4], f32, kind="ExternalInput").ap()
    hpar_d = nc.dram_tensor("hpar", [NH, 4], f32, kind="ExternalInput").ap()
    patm_d = nc.dram_tensor("patm", [128, NH * 256], f32, kind="ExternalInput").ap()
    psel_d = nc.dram_tensor("psel", [NH, NH * 256], f32, kind="ExternalInput").ap()

    mnew_d = nc.dram_tensor("mnew", [nc_rows, D], f32, kind="ExternalOutput").ap()
    wout_d = nc.dram_tensor("wout", [NH, nc_rows], f32, kind="ExternalOutput").ap()
    read_d = nc.dram_tensor("readings", [D, R], f32, kind="ExternalOutput").ap()

    rg = [list(range(NCORES))]

    with tile.TileContext(nc) as tc:
        with (
            tc.tile_pool(name="const", bufs=1) as cp,
            tc.tile_pool(name="mres", bufs=1) as mres,
            tc.tile_pool(name="warr", bufs=1) as wa,
            tc.tile_pool(name="dram", bufs=1, space="DRAM") as dp,
            tc.tile_pool(name="smallp", bufs=1, space="PSUM") as sp,
        ):
            # ---------------- constants ----------------
            kb = cp.tile([D, NH], f32)
            nc.sync.dma_start(kb[:], kb_d[:])
            c3 = cp.tile([128, 128 * NH], f32)
            nc.sync.dma_start(c3[:], c3_d[:])
            e4 = cp.tile([128, TILE], f32)
            nc.sync.dma_start(e4[:], e4_d[:])
            eblk = cp.tile([4, TILE], f32r)
            nc.sync.dma_start(eblk[:], eblk_d[:])
            ablk = cp.tile([4, TILE], f32r)
            nc.sync.dma_start(ablk[:], ablk_d[:])
            i128 = cp.tile([128, 128], f32)
            nc.sync.dma_start(i128[:], i128_d[:])
            ones4 = cp.tile([128, 4], f32)
            nc.sync.dma_start(ones4[:], ones4_d[:])
            hpar = cp.tile([NH, 4], f32)
            nc.sync.dma_start(hpar[:], hpar_d[:])
            patm = cp.tile([128, NH * 256], f32)
            nc.sync.dma_start(patm[:], patm_d[:])
            psel = cp.tile([NH, NH * 256], f32)
            nc.sync.dma_start(psel[:], psel_d[:])
            mhalo = cp.tile([2, D], f32)
            nc.sync.dma_start(mhalo[:], mhalo_d[:])
            inithalo = cp.tile([NH, 2], f32)
            nc.sync.dma_start(inithalo[:], inithalo_d[:])

            # ---------------- w-layout arrays ----------------
            simS = wa.tile([128, NH * hblk], f32)
            nsqS = wa.tile([128, hblk], f32)
            initS = wa.tile([128, NH * hblk], f32)
            expC = wa.tile([128, NH * hblk], f32)
            expI = wa.tile([128, NH * hblk], f32)
            d2S = wa.tile([128, NH * hblk], f32)
            wg = wa.tile([128, NH * hblk], f32)
            wt = wa.tile([128, NH * hblk], f32)
            wp = wa.tile([128, NH * hblk], f32)
            wn = wa.tile([128, NH * hblk], f32)
            ar1 = wa.tile([128, 8], f32)
            ar2 = wa.tile([128, 8], f32)

            nc.vector.memset(ar1[:], 0.0)
            nc.vector.memset(ar2[:], 0.0)

            m_tiles = []
            for t in range(nt):
                mt = mres.tile([128, TILE], f32, tag=f"m{t}", name=f"mt{t}")
                m_tiles.append(mt)

            # ---------------- init repack + exp ----------------
            with tc.tile_pool(name="ini", bufs=2) as ip, \
                 tc.tile_pool(name="inip", bufs=2, space="PSUM") as ipp:
                for h in range(NH):
                    for half in range(2):
                        v = ip.tile([128, hw2], f32, tag="v")
                        src = init_d[h, half * (128 * hw2):(half + 1) * (128 * hw2)]
                        nc.sync.dma_start(v[:], src.rearrange("(i j) -> i j", i=128))
                        vt = ipp.tile([128, 128], f32, tag="vt")
                        # v is [128, hw2]; transpose hw2x? chunks of 128
                        for b in range(hw2 // 128 if hw2 >= 128 else 1):
                            pass
                        # hw2 == 128 for full size; handle general case below
                        assert hw2 == 128, "init repack assumes NC=32768"
                        nc.tensor.transpose(vt[:], v[:], i128[:])
                        nc.scalar.copy(initS[:, h * hblk + half * 128:
                                             h * hblk + (half + 1) * 128], vt[:])
                for h in range(NH):
                    nc.scalar.activation(
                        expI[:, h * hblk:(h + 1) * hblk],
                        initS[:, h * hblk:(h + 1) * hblk],
                        AF.Exp, accum_out=ar1[:, 3 + h:4 + h])

            # ---------------- halo content path ----------------
            mhT_p = sp.tile([128, 2], f32, tag="sp", bufs=2)
            nc.tensor.transpose(mhT_p[:], mhalo[:], i128[0:2, 0:2])
            mhT = cp.tile([128, 2], f32)
            nc.vector.tensor_copy(mhT[:], mhT_p[:])
            sqh = cp.tile([128, 2], f32)
            nc.gpsimd.tensor_tensor(sqh[:], mhT[:], mhT[:], ALU.mult)
            nsqh_p = sp.tile([NH, 2], f32, tag="sp", bufs=2)
            nc.tensor.matmul(nsqh_p[:], ones4[:, 0:NH], sqh[:], start=True, stop=True)
            simh_p = sp.tile([NH, 2], f32, tag="sp", bufs=2)
            nc.tensor.matmul(simh_p[:], kb[:], mhT[:], start=True, stop=True)
            # halo pipeline on [NH,2] tiles; hpar col0=beta col1=nk2
            d2h = cp.tile([NH, 2], f32)
            nc.vector.tensor_scalar_mul(d2h[:], nsqh_p[:], hpar[:, 1:2])
            irh = cp.tile([NH, 2], f32)
            nc.scalar.sqrt(irh[:], d2h[:])
            nc.vector.reciprocal(irh[:], irh[:])
            xh = cp.tile([NH, 2], f32)
            nc.vector.scalar_tensor_tensor(
                xh[:], simh_p[:], hpar[:, 0:1], irh[:], ALU.mult, ALU.mult)
            ech = cp.tile([NH, 2], f32)
            nc.scalar.activation(ech[:], xh[:], AF.Exp)
            eih = cp.tile([NH, 2], f32)
            nc.scalar.activation(eih[:], inithalo[:], AF.Exp)

            # ---------------- phase A: stream M ----------------
            with tc.tile_pool(name="pa", bufs=3) as pa, \
                 tc.tile_pool(name="pap", bufs=2, space="PSUM") as pap, \
                 tc.tile_pool(name="pap2", bufs=2, space="PSUM") as pap2:
                for t in range(nt):
                    mt = m_tiles[t]
                    src = m_d[t * TILE:(t + 1) * TILE, :]
                    nc.sync.dma_start(
                        mt[:].rearrange("p (c d) -> p c d", c=4),
                        src.rearrange("(c p) d -> p c d", p=128))
                    mT_p = pap.tile([128, TILE], f32, tag="mT")
                    for cch in range(4):
                        nc.tensor.transpose(
                            mT_p[:, cch * 128:(cch + 1) * 128],
                            mt[:, cch * 128:(cch + 1) * 128], i128[:])
                    mT = pa.tile([128, TILE], f32, tag="mTs")
                    nc.scalar.copy(mT[:], mT_p[:])
                    # nsq: square (ACT/GP split) + 3D free-reduce on DVE
                    sq = pa.tile([128, TILE], f32, tag="sq")
                    if t % 2 == 0:
                        nc.scalar.square(sq[:], mt[:])
                    else:
                        nc.gpsimd.tensor_tensor(sq[:], mt[:], mt[:], ALU.mult)
                    nc.vector.tensor_reduce(
                        nsqS[:, 4 * t:4 * t + 4],
                        sq[:].rearrange("p (c d) -> p c d", c=4),
                        AX.X, ALU.add)
                    # sim: per chunk, MiT as stationary, KB moving
                    sim_p = pap2.tile([128, 4 * NH], f32, tag="simp")
                    for cch in range(4):
                        nc.tensor.matmul(
                            sim_p[:, cch * NH:(cch + 1) * NH],
                            mT[:, cch * 128:(cch + 1) * 128], kb[:],
                            start=True, stop=True)
                    nc.vector.tensor_copy(
                        simS[:, :].rearrange("p (h f) -> p h f", h=NH)
                                  [:, :, 4 * t:4 * t + 4],
                        sim_p[:].rearrange("p (c h) -> p h c", c=4))

            # ---------------- softmax prep + AR1 ----------------
            for h in range(NH):
                nc.vector.tensor_scalar_mul(
                    d2S[:, h * hblk:(h + 1) * hblk], nsqS[:],
                    float((heads[h]["k"].astype(np.float64) ** 2).sum()))
            nc.scalar.sqrt(d2S[:], d2S[:])
            nc.vector.reciprocal(d2S[:], d2S[:])
            for h in range(NH):
                xs = expC[:, h * hblk:(h + 1) * hblk]
                nc.vector.scalar_tensor_tensor(
                    xs, simS[:, h * hblk:(h + 1) * hblk], float(heads[h]["beta"]),
                    d2S[:, h * hblk:(h + 1) * hblk], ALU.mult, ALU.mult)
                nc.scalar.activation(xs, xs, AF.Exp, accum_out=ar1[:, h:h + 1])

            ar1_in = dp.tile([128, 8], f32)
            ar1_out = dp.tile([128, 8], f32, addr_space="Shared")
            nc.sync.dma_start(ar1_in[:], ar1[:])
            nc.gpsimd.collective_compute(
                "AllReduce", mybir.AluOpType.add, replica_groups=rg,
                ins=[ar1_in[:].opt()], outs=[ar1_out[:].opt()])
            ar1r = wa.tile([128, 8], f32)
            nc.sync.dma_start(ar1r[:], ar1_out[:])

            # global sums + scales
            s1_p = sp.tile([1, 8], f32, tag="sp", bufs=2)
            nc.tensor.matmul(s1_p[:], ones4[:, 0:1], ar1r[:], start=True, stop=True)
            s1 = wa.tile([1, 8], f32)
            nc.vector.tensor_copy(s1[:], s1_p[:])
            inv1 = wa.tile([1, 8], f32)
            nc.vector.reciprocal(inv1[:, 0:6], s1[:, 0:6])
            sc = wa.tile([1, 8], f32)
            for h in range(NH):
                nc.vector.tensor_scalar_mul(
                    sc[:, h:h + 1], inv1[:, h:h + 1], float(heads[h]["gg"]))
                nc.vector.tensor_scalar_mul(
                    sc[:, 3 + h:4 + h], inv1[:, 3 + h:4 + h],
                    float(1.0 - heads[h]["gg"]))
            scb = wa.tile([128, 8], f32)
            nc.gpsimd.partition_broadcast(scb[:], sc[:])
            # transposed scales for halo ([NH,1] at partitions 0..2)
            scT_p = sp.tile([NH, 2], f32, tag="sp", bufs=2)
            nc.tensor.transpose(scT_p[:, 0:1], sc[:, 0:NH], i128[0:1, 0:1])
            nc.tensor.transpose(scT_p[:, 1:2], sc[:, 3:3 + NH], i128[0:1, 0:1])
            scT = wa.tile([NH, 2], f32)
            nc.vector.tensor_copy(scT[:], scT_p[:])

            # ---------------- interpolate ----------------
            for h in range(NH):
                hsl = slice(h * hblk, (h + 1) * hblk)
                nc.vector.tensor_scalar_mul(
                    wg[:, hsl], expI[:, hsl], scb[:, 3 + h:4 + h])
                nc.vector.scalar_tensor_tensor(
                    wg[:, hsl], expC[:, hsl], scb[:, h:h + 1], wg[:, hsl],
                    ALU.mult, ALU.add)
            # halo wg
            wgh = wa.tile([NH, 2], f32)
            nc.vector.tensor_scalar_mul(wgh[:], eih[:], scT[:, 1:2])
            nc.vector.scalar_tensor_tensor(
                wgh[:], ech[:], scT[:, 0:1], wgh[:], ALU.mult, ALU.add)

            # ---------------- shift + sharpen ----------------
            with tc.tile_pool(name="wtp", bufs=2, space="PSUM") as wtp:
                for h in range(NH):
                    hsl = slice(h * hblk, (h + 1) * hblk)
                    s0 = float(heads[h]["s"][0])
                    s2 = float(heads[h]["s"][2])
                    wt_p = wtp.tile([128, hblk], f32, tag="wtp")
                    nc.tensor.matmul(
                        wt_p[:], c3[:, h * 128:(h + 1) * 128], wg[:, hsl],
                        start=True, stop=False)
                    # boundary patches, accumulated into the same PSUM:
                    # row 0, f>=1: += s2*wg[127, f-1]   (P2 = s2*E[127->0])
                    nc.tensor.matmul(
                        wt_p[:, 1:hblk], patm[:, h * 256:h * 256 + 128],
                        wg[:, h * hblk:(h + 1) * hblk - 1],
                        start=False, stop=False)
                    # row 127, f<=hblk-2: += s0*wg[0, f+1]   (P0 = s0*E[0->127])
                    nc.tensor.matmul(
                        wt_p[:, 0:hblk - 1], patm[:, h * 256 + 128:h * 256 + 256],
                        wg[:, h * hblk + 1:(h + 1) * hblk],
                        start=False, stop=False)
                    # corners from halo: row0,f=0 += s2*wgh[h,0]; row127,f=last += s0*wgh[h,1]
                    nc.tensor.matmul(
                        wt_p[:, 0:1], psel[:, h * 256:h * 256 + 128],
                        wgh[:, 0:1], start=False, stop=False)
                    nc.tensor.matmul(
                        wt_p[:, hblk - 1:hblk], psel[:, h * 256 + 128:h * 256 + 256],
                        wgh[:, 1:2], start=False, stop=True)
                    nc.scalar.copy(wt[:, hsl], wt_p[:])
                    # sharpen: wp = exp(gamma * ln(wt))
                    nc.scalar.activation(wp[:, hsl], wt[:, hsl], AF.Ln)
                    nc.scalar.activation(wp[:, hsl], wp[:, hsl], AF.Exp,
                                         scale=float(heads[h]["gamma"]))
                    nc.vector.tensor_reduce(
                        ar2[:, 2 + h:3 + h], wp[:, hsl], AX.X, ALU.add)

            # ---------------- readings (unnormalized) + AR2 ----------------
            with tc.tile_pool(name="rdp", bufs=1, space="PSUM") as rdp:
                rd_p = rdp.tile([128, 2], f32)
                wp3 = wp[:].rearrange("p (h f) -> p h f", h=NH)
                nchunks = 4 * nt
                for i in range(nchunks):
                    t, cch = i // 4, i % 4
                    nc.tensor.matmul(
                        rd_p[:], m_tiles[t][:, cch * 128:(cch + 1) * 128],
                        wp3[:, 0:2, i:i + 1],
                        start=(i == 0), stop=(i == nchunks - 1))
                nc.vector.tensor_copy(ar2[:, 0:2], rd_p[:])

            ar2_in = dp.tile([128, 8], f32)
            ar2_out = dp.tile([128, 8], f32, addr_space="Shared")
            nc.sync.dma_start(ar2_in[:], ar2[:])
            nc.gpsimd.collective_compute(
                "AllReduce", mybir.AluOpType.add, replica_groups=rg,
                ins=[ar2_in[:].opt()], outs=[ar2_out[:].opt()])
            ar2r = wa.tile([128, 8], f32)
            nc.sync.dma_start(ar2r[:], ar2_out[:])

            s2_p = sp.tile([1, 8], f32, tag="sp", bufs=2)
            nc.tensor.matmul(s2_p[:], ones4[:, 0:1], ar2r[:], start=True, stop=True)
            s2s = wa.tile([1, 8], f32)
            nc.vector.tensor_copy(s2s[:], s2_p[:])
            inv2 = wa.tile([1, 4], f32)
            nc.vector.tensor_scalar_add(inv2[:, 0:3], s2s[:, 2:5], EPS)
            nc.vector.reciprocal(inv2[:, 0:3], inv2[:, 0:3])
            inv2b = wa.tile([128, 4], f32)
            nc.gpsimd.partition_broadcast(inv2b[:], inv2[:])

            # readings final
            rfin = wa.tile([128, R], f32)
            for h in range(R):
                nc.vector.tensor_scalar_mul(
                    rfin[:, h:h + 1], ar2r[:, h:h + 1], inv2b[:, h:h + 1])
            nc.sync.dma_start(read_d[:], rfin[:])

            # normalize weights; write wout
            with tc.tile_pool(name="wop", bufs=2, space="PSUM") as wop, \
                 tc.tile_pool(name="wos", bufs=2) as wos:
                for h in range(NH):
                    hsl = slice(h * hblk, (h + 1) * hblk)
                    nc.vector.tensor_scalar_mul(
                        wn[:, hsl], wp[:, hsl], inv2b[:, h:h + 1])
                    for half in range(2):
                        vo_p = wop.tile([128, 128], f32, tag="vop")
                        nc.tensor.transpose(
                            vo_p[:],
                            wn[:, h * hblk + half * 128:h * hblk + (half + 1) * 128],
                            i128[:])
                        vo = wos.tile([128, 128], f32, tag="vo")
                        nc.scalar.copy(vo[:], vo_p[:])
                        dst = wout_d[h, half * (128 * hw2):(half + 1) * (128 * hw2)]
                        nc.sync.dma_start(
                            dst.rearrange("(i j) -> i j", i=128), vo[:])

            # ---------------- phase B: erase/add write ----------------
            with tc.tile_pool(name="pb", bufs=3) as pb, \
                 tc.tile_pool(name="pbw", bufs=3) as pbw, \
                 tc.tile_pool(name="pbp", bufs=2, space="PSUM") as pbp, \
                 tc.tile_pool(name="pbp2", bufs=2, space="PSUM") as pbp2, \
                 tc.tile_pool(name="pbp3", bufs=2, space="PSUM") as pbp3:
                junk = wa.tile([128, 1], f32)
                for t in range(nt):
                    # w columns for this tile's 4 chunks -> [4,128] at partition 0
                    wt4_p = pbp3.tile([4, 128], f32, tag="wt4p")
                    nc.tensor.transpose(
                        wt4_p[:], wn[:, R * hblk + 4 * t:R * hblk + 4 * t + 4],
                        i128[:])
                    wT4 = pbw.tile([4, 128], f32r, tag="wt4")
                    nc.vector.tensor_copy(wT4[:], wt4_p[:])
                    w4e_p = pbp.tile([128, TILE], f32, tag="w4e")
                    nc.tensor.matmul(
                        w4e_p[:], wT4[:], eblk[:], start=True, stop=True)
                    w4a_p = pbp2.tile([128, TILE], f32, tag="w4a")
                    nc.tensor.matmul(
                        w4a_p[:], wT4[:], ablk[:], start=True, stop=True)
                    t1 = pb.tile([128, TILE], f32, tag="t1")
                    nc.vector.affine_mul_reduce(
                        t1[:], junk[:], w4e_p[:], m_tiles[t][:], -1.0, 1.0)
                    w4a = pb.tile([128, TILE], f32, tag="w4a_s")
                    nc.scalar.copy(w4a[:], w4a_p[:])
                    mnew = pb.tile([128, TILE], f32, tag="mnew")
                    nc.gpsimd.tensor_tensor(mnew[:], t1[:], w4a[:], ALU.add)
                    dst = mnew_d[t * TILE:(t + 1) * TILE, :]
                    nc.sync.dma_start(
                        dst.rearrange("(c p) d -> p c d", p=128),
                        mnew[:].rearrange("p (c d) -> p c d", c=4))

    nc.compile()
    return nc


# ------------------------------------------------------------------ driver
def _build_core_inputs(inp, c, heads, e, a, nc_rows=NC, ncores=NCORES):
    M = np.asarray(inp["M"], np.float32)
    init = np.concatenate([np.asarray(inp["rinit"], np.float32),
                           np.asarray(inp["winit"], np.float32)], axis=0)
    n = M.shape[0]
    kbm = np.stack([heads[h]["k"] for h in range(NH)], axis=1).astype(np.float32)
    c3 = np.zeros((128, 128 * NH), np.float32)
    for h in range(NH):
        s = heads[h]["s"]
        blk = np.zeros((128, 128), np.float32)
        for m in range(128):
            if m + 1 < 128:
                blk[m + 1, m] = s[0]
            blk[m, m] += s[1]
            if m - 1 >= 0:
                blk[m - 1, m] = s[2]
        c3[:, h * 128:(h + 1) * 128] = blk
    e4 = np.tile(e, (128, 4)).astype(np.float32)
    eblk = np.zeros((4, TILE), np.float32)
    ablk = np.zeros((4, TILE), np.float32)
    for k in range(4):
        eblk[k, k * 128:(k + 1) * 128] = e
        ablk[k, k * 128:(k + 1) * 128] = a
    i128 = np.eye(128, dtype=np.float32)
    ones4 = np.ones((128, 4), np.float32)
    hpar = np.zeros((NH, 4), np.float32)
    for h in range(NH):
        hpar[h, 0] = heads[h]["beta"]
        hpar[h, 1] = (heads[h]["k"].astype(np.float64) ** 2).sum()
    patm = np.zeros((128, NH * 256), np.float32)
    psel = np.zeros((NH, NH * 256), np.float32)
    for h in range(NH):
        s = heads[h]["s"]
        patm[127, h * 256 + 0] = s[2]          # P2: k=127 -> m=0
        patm[0, h * 256 + 128 + 127] = s[0]    # P0: k=0 -> m=127
        psel[h, h * 256 + 0] = s[2]            # selL: out row 0
        psel[h, h * 256 + 128 + 127] = s[0]    # selR: out row 127

    maps = []
    for ci in range(ncores):
        r0 = ci * nc_rows
        maps.append({
            "m": np.ascontiguousarray(M[r0:r0 + nc_rows]),
            "mhalo": np.ascontiguousarray(
                np.stack([M[(r0 - 1) % n], M[(r0 + nc_rows) % n]])),
            "init": np.ascontiguousarray(init[:, r0:r0 + nc_rows]),
            "inithalo": np.ascontiguousarray(
                np.stack([init[:, (r0 - 1) % n], init[:, (r0 + nc_rows) % n]],
                         axis=1)),
            "kb": kbm, "c3": c3, "e4": e4, "eblk": eblk, "ablk": ablk,
            "i128": i128, "ones4": ones4, "hpar": hpar, "patm": patm,
            "psel": psel,
        })
    return maps


def run_on_hw(inp, trace=False, trace_kwargs=None):
    import concourse.bacc as bacc
    from concourse.bass_utils import run_bass_kernel_spmd

    c, heads, e, a = host_params(inp)
    nc = bacc.Bacc("TRN2", target_bir_lowering=False, debug=False,
                   num_devices=NCORES)
    build(nc, NC, heads, e, a)
    maps = _build_core_inputs(inp, c, heads, e, a)
    res = run_bass_kernel_spmd(
        nc, maps, list(range(NCORES)), trace=trace,
        **(trace_kwargs or {}))
    return _assemble(inp, res.results, c, heads), res


def _assemble(inp, results, c, heads):
    read_w = np.concatenate([r["wout"][:R] for r in results], axis=1)
    write_w = np.concatenate([r["wout"][R:] for r in results], axis=1)
    M_new = np.concatenate([r["mnew"] for r in results], axis=0)
    readings = np.ascontiguousarray(results[0]["readings"].T)      # [R, D]
    Wo = np.asarray(inp["Wo"], np.float32)
    bo = np.asarray(inp["bo"], np.float32)
    out = (Wo @ np.concatenate([c, readings.reshape(-1)]) + bo).astype(np.float32)
    return out, readings, read_w, write_w, M_new


def kernel(**inputs):
    outs, _ = run_on_hw(inputs)
    return outs
